# revision 11
# baseline (speedup 1.0000x reference)
"""Trainium2 kernel for CompactBilinearLayer (count-sketch bilinear pooling).

Math: reference computes y = l2norm(signed_sqrt(sum_hw Re IFFT(FFT(x@M1)*FFT(x@M2)))).
Since M1/M2 are count-sketch matrices (one +-1 per row), FFT(x@M1) == x @ A1 with
A1[c,k] = s1[c] * exp(-2pi i h1[c] k / P) — a dense [512, K] matrix computable on the
host from M1 in O(C*K). The IFFT is linear, so the spatial sum moves before it.
Hermitian symmetry means only k = 0..4096 are needed.  Per core (4 batch elements,
784 spatial positions — fully batch-local, no collectives):
  A: P1/P2 projections = A^T @ x^T, single-pass bf16 matmuls; per-component
     [128,784] PSUM tiles with bank-aligned 512+272 splits; Act evacuates each
     component to SBUF so PSUM recycles fast and the PE stays fed
  B: S[k,b] = sum_t (P1*P2) per batch via fused DVE scalar_tensor_tensor
     (product+reduce in one op), operands all-SBUF
  C: IFFT via two-step factorization n=64q+s: GpSimd computes the twiddle
     products (f32r), PE accumulates 4 f32r matmuls over k%128 into psy.
     Stage-C matmuls are emitted 2 iterations late so the in-order PE queue
     never waits on the DVE->GpSimd chain.
  D: signed sqrt + per-batch L2 norm + store
"""
import numpy as np

P = 8192
C = 512
FT = 33            # frequency tiles of 128 -> 4224 slots >= 4097
NSLOT = FT * 128
NCORES = 8
BPC = 4            # batch elems per core
HW = 196           # spatial positions per batch elem
T = BPC * HW       # 784 positions per core
B = 32

_CACHE = {}


def _build_program():
    import concourse.bass as bass
    import concourse.tile as tile
    from concourse import bacc, mybir

    f32 = mybir.dt.float32
    f32r = mybir.dt.float32r
    bf16 = mybir.dt.bfloat16
    nc = bacc.Bacc("TRN2", target_bir_lowering=False, debug=False,
                   num_devices=NCORES)

    a_d = nc.dram_tensor("a", [FT, C, 512], bf16, kind="ExternalInput").ap()
    x_d = nc.dram_tensor("x", [C, T], bf16, kind="ExternalInput").ap()
    cphi_d = nc.dram_tensor("cphi", [FT, 128, 64], f32, kind="ExternalInput").ap()
    sphi_d = nc.dram_tensor("sphi", [FT, 128, 64], f32, kind="ExternalInput").ap()
    cosa_d = nc.dram_tensor("cosa", [128, 128], f32r, kind="ExternalInput").ap()
    ncosa_d = nc.dram_tensor("ncosa", [128, 128], f32r, kind="ExternalInput").ap()
    nsina_d = nc.dram_tensor("nsina", [128, 128], f32r, kind="ExternalInput").ap()
    y_d = nc.dram_tensor("y", [BPC, P], f32, kind="ExternalOutput").ap()

    mult = mybir.AluOpType.mult
    bypass = mybir.AluOpType.bypass
    Act = mybir.ActivationFunctionType

    with tile.TileContext(nc) as tc:
        with (
            tc.tile_pool(name="const", bufs=1) as const,
            tc.tile_pool(name="apool", bufs=3) as apool,
            tc.tile_pool(name="pst", bufs=3, space="PSUM") as pstpool,
            tc.tile_pool(name="psyp", bufs=1, space="PSUM") as psypool,
            tc.tile_pool(name="scr", bufs=3) as scr,
            tc.tile_pool(name="uv", bufs=4) as uvpool,
        ):
            x_sb = const.tile([128, 4, T], bf16)
            nc.sync.dma_start(x_sb[:], x_d.rearrange("(ck p) t -> p ck t", p=128))
            cphi_sb = const.tile([128, FT, 64], f32)
            nc.sync.dma_start(cphi_sb[:], cphi_d.rearrange("kt p s -> p kt s"))
            sphi_sb = const.tile([128, FT, 64], f32)
            nc.sync.dma_start(sphi_sb[:], sphi_d.rearrange("kt p s -> p kt s"))
            cosa_sb = const.tile([128, 128], f32r)
            nc.sync.dma_start(cosa_sb[:], cosa_d)
            ncosa_sb = const.tile([128, 128], f32r)
            nc.sync.dma_start(ncosa_sb[:], ncosa_d)
            nsina_sb = const.tile([128, 128], f32r)
            nc.sync.dma_start(nsina_sb[:], nsina_d)
            ones_sb = const.tile([128, 1], f32)
            nc.vector.memset(ones_sb[:], 1.0)
            sre_sb = const.tile([128, FT * 4], f32)
            sim_sb = const.tile([128, FT * 4], f32)
            sA_sb = const.tile([128, FT * 4], f32)
            sB_sb = const.tile([128, FT * 4], f32)
            sC_sb = const.tile([128, FT * 4], f32)
            sD_sb = const.tile([128, FT * 4], f32)

            psy = psypool.tile([128, BPC * 64], f32, tag="psy")

            def emit_stage_c(ft, us):
                u1, u2, v1, v2 = us
                nc.tensor.matmul(psy[:], cosa_sb[:],
                                 u1[:].rearrange("p b s -> p (b s)"),
                                 start=(ft == 0), stop=False)
                nc.tensor.matmul(psy[:], ncosa_sb[:],
                                 u2[:].rearrange("p b s -> p (b s)"),
                                 start=False, stop=False)
                nc.tensor.matmul(psy[:], nsina_sb[:],
                                 v1[:].rearrange("p b s -> p (b s)"),
                                 start=False, stop=False)
                nc.tensor.matmul(psy[:], nsina_sb[:],
                                 v2[:].rearrange("p b s -> p (b s)"),
                                 start=False, stop=(ft == FT - 1))

            pend = {}
            for ft in range(FT):
                a_t = apool.tile([128, 4, 512], bf16, tag="a")
                nc.sync.dma_start(
                    a_t[:], a_d[ft].rearrange("(ck p) m -> p ck m", p=128)
                )
                psm = {}
                cpy = {}
                # components 2,3 first: Act evacuates them to SBUF (the stt
                # in1 operand); 0,1 stay in PSUM so each stt uses only one
                # SBUF read port and does not contend with GpSimd's port
                for m in (2, 3, 0, 1):
                    msl = slice(m * 128, (m + 1) * 128)
                    ps = pstpool.tile([128, T], f32, tag="pst",
                                      name=f"ps{m}_{ft}")
                    # bank-aligned output splits: 512 (bank 0), 272 (bank 1)
                    for c0, cn in ((0, 512), (512, T - 512)):
                        for ck in range(4):
                            nc.tensor.matmul(
                                ps[:, c0:c0 + cn],
                                a_t[:, ck, msl],
                                x_sb[:, ck, c0:c0 + cn],
                                start=(ck == 0),
                                stop=(ck == 3),
                            )
                    psm[m] = ps
                    if m in (2, 3):
                        c_m = scr.tile([128, T], f32, tag=f"c{m}",
                                       name=f"c{m}_{ft}")
                        nc.scalar.activation(c_m[:], ps[:], Act.Copy)
                        cpy[m] = c_m

                # A=sum p0*p2, B=sum p1*p3, C=sum p0*p3, D=sum p1*p2.
                # All ps0-reading ops first so ps0's PSUM ring slot frees
                # before the next tile's matmuls need it.
                for tg, (pa, cb, dst) in (
                    ("sc0", (psm[0], cpy[2], sA_sb)),
                    ("sc2", (psm[0], cpy[3], sC_sb)),
                    ("sc1", (psm[1], cpy[3], sB_sb)),
                    ("sc3", (psm[1], cpy[2], sD_sb)),
                ):
                    for bl in range(BPC):
                        idx = ft * 4 + bl
                        seg = slice(bl * HW, (bl + 1) * HW)
                        sc = scr.tile([128, HW], f32, tag=tg,
                                      name=f"{tg}_{ft}_{bl}")
                        nc.vector.scalar_tensor_tensor(
                            sc[:], pa[:, seg], 1.0, cb[:, seg],
                            bypass, mult,
                            accum_out=dst[:, idx:idx + 1])
                # ReS = A - B, ImS = C + D for this ft's 4 batch slots
                fsl = slice(ft * 4, (ft + 1) * 4)
                nc.vector.tensor_sub(sre_sb[:, fsl], sA_sb[:, fsl],
                                     sB_sb[:, fsl])
                nc.vector.tensor_add(sim_sb[:, fsl], sC_sb[:, fsl],
                                     sD_sb[:, fsl])

                # twiddle products on GpSimd (u = phi * S, broadcast both ways)
                u1 = uvpool.tile([128, BPC, 64], f32r, tag="u1", name=f"u1_{ft}")
                u2 = uvpool.tile([128, BPC, 64], f32r, tag="u2", name=f"u2_{ft}")
                v1 = uvpool.tile([128, BPC, 64], f32r, tag="v1", name=f"v1_{ft}")
                v2 = uvpool.tile([128, BPC, 64], f32r, tag="v2", name=f"v2_{ft}")
                cphb = cphi_sb[:, ft, :][:, None, :].broadcast_to([128, BPC, 64])
                sphb = sphi_sb[:, ft, :][:, None, :].broadcast_to([128, BPC, 64])
                sreb = sre_sb[:, fsl][:, :, None].broadcast_to([128, BPC, 64])
                simb = sim_sb[:, fsl][:, :, None].broadcast_to([128, BPC, 64])
                nc.gpsimd.tensor_tensor(u1[:], cphb, sreb, op=mult)
                nc.gpsimd.tensor_tensor(u2[:], sphb, simb, op=mult)
                nc.gpsimd.tensor_tensor(v1[:], sphb, sreb, op=mult)
                nc.gpsimd.tensor_tensor(v2[:], cphb, simb, op=mult)
                pend[ft] = (u1, u2, v1, v2)

                # emit IFFT matmuls 2 iterations late to keep the PE queue fed
                if ft >= 2:
                    emit_stage_c(ft - 2, pend.pop(ft - 2))
            emit_stage_c(FT - 2, pend.pop(FT - 2))
            emit_stage_c(FT - 1, pend.pop(FT - 1))

            # ---- stage D: signed sqrt, per-batch l2 norm, store ----
            absy = const.tile([128, BPC * 64], f32)
            nc.scalar.activation(absy[:], psy[:], Act.Abs)
            sqy = const.tile([128, BPC * 64], f32)
            nc.scalar.activation(sqy[:], absy[:], Act.Sqrt)
            sgn = const.tile([128, BPC * 64], f32)
            nc.scalar.activation(sgn[:], psy[:], Act.Sign)
            ys = const.tile([128, BPC * 64], f32)
            nc.vector.tensor_mul(ys[:], sqy[:], sgn[:])

            # norm^2 per batch = sum_p y^2 = sum_p |Y|  (Y = pre-sqrt value)
            psn = pstpool.tile([128, BPC * 64], f32, tag="pst", name="psn")
            nc.tensor.matmul(psn[0:1, :], ones_sb[:], absy[:],
                             start=True, stop=True)
            nsq = const.tile([1, BPC], f32)
            nc.vector.reduce_sum(
                out=nsq[:],
                in_=psn[0:1, :].rearrange("p (b s) -> p b s", b=BPC),
                axis=mybir.AxisListType.X,
            )
            nc.vector.tensor_scalar_max(nsq[:], nsq[:], 1e-10)
            sqn = const.tile([1, BPC], f32)
            nc.scalar.activation(sqn[:], nsq[:], Act.Sqrt)
            invn = const.tile([1, BPC], f32)
            nc.vector.reciprocal(invn[:], sqn[:])

            onesrow = const.tile([1, 128], f32)
            nc.vector.memset(onesrow[:], 1.0)
            psb = pstpool.tile([128, BPC], f32, tag="pst", name="psb")
            nc.tensor.matmul(psb[:, 0:BPC], onesrow[0:1, :], invn[0:1, :],
                             start=True, stop=True)
            inv_b = psb[:, 0:BPC][:, :, None].broadcast_to([128, BPC, 64])
            fin = const.tile([128, BPC * 64], f32)
            nc.vector.tensor_tensor(
                fin[:].rearrange("p (b s) -> p b s", b=BPC),
                ys[:].rearrange("p (b s) -> p b s", b=BPC),
                inv_b,
                op=mult,
            )
            for b in range(BPC):
                nc.sync.dma_start(
                    y_d[b].rearrange("(q s) -> q s", q=128),
                    fin[:, b * 64:(b + 1) * 64],
                )

    nc.compile()
    return nc


def _to_bf16(a):
    import ml_dtypes
    return np.asarray(a, np.float32).astype(ml_dtypes.bfloat16)


def _host_prep(x, M1, M2):
    x = np.ascontiguousarray(np.asarray(x, np.float32))
    M1 = np.asarray(M1, np.float32)
    M2 = np.asarray(M2, np.float32)

    h1 = np.argmax(np.abs(M1), axis=1)
    s1 = M1[np.arange(C), h1].astype(np.float64)
    h2 = np.argmax(np.abs(M2), axis=1)
    s2 = M2[np.arange(C), h2].astype(np.float64)

    k = np.arange(NSLOT, dtype=np.float64)
    valid = k <= P // 2
    ang1 = 2 * np.pi * np.outer(h1.astype(np.float64), k) / P
    ang2 = 2 * np.pi * np.outer(h2.astype(np.float64), k) / P
    # a[ft, c, m*128 + j]: m in (A1re, A1im, A2re, A2im), freq = ft*128 + j
    a = np.empty((FT, C, 512), np.float32)
    a1re = (s1[:, None] * np.cos(ang1) * valid).astype(np.float32)
    a1im = (-s1[:, None] * np.sin(ang1) * valid).astype(np.float32)
    a2re = (s2[:, None] * np.cos(ang2) * valid).astype(np.float32)
    a2im = (-s2[:, None] * np.sin(ang2) * valid).astype(np.float32)
    for ft in range(FT):
        ksl = slice(ft * 128, (ft + 1) * 128)
        a[ft, :, 0:128] = a1re[:, ksl]
        a[ft, :, 128:256] = a1im[:, ksl]
        a[ft, :, 256:384] = a2re[:, ksl]
        a[ft, :, 384:512] = a2im[:, ksl]

    w = np.where(valid, 2.0 / P, 0.0)
    w[0] = 1.0 / P
    w[P // 2] = 1.0 / P
    s_idx = np.arange(64, dtype=np.float64)
    phi = 2 * np.pi * np.outer(k, s_idx) / P
    cphi = (w[:, None] * np.cos(phi)).astype(np.float32).reshape(FT, 128, 64)
    sphi = (w[:, None] * np.sin(phi)).astype(np.float32).reshape(FT, 128, 64)

    km = np.arange(128, dtype=np.float64)
    alpha = 2 * np.pi * np.outer(km, km) / 128
    cosa = np.cos(alpha).astype(np.float32)
    nsina = (-np.sin(alpha)).astype(np.float32)

    xt = np.ascontiguousarray(x.reshape(B * HW, C).T)  # [C, 6272]

    return _to_bf16(a), cphi, sphi, cosa, -cosa, nsina, _to_bf16(xt)


def _make_in_maps(x, M1, M2):
    a, cphi, sphi, cosa, ncosa, nsina, xt = _host_prep(x, M1, M2)
    in_maps = []
    for r in range(NCORES):
        in_maps.append({
            "a": a,
            "x": np.ascontiguousarray(xt[:, r * T:(r + 1) * T]),
            "cphi": cphi,
            "sphi": sphi,
            "cosa": cosa,
            "ncosa": ncosa,
            "nsina": nsina,
        })
    return in_maps


def kernel(x, M1, M2):
    from concourse.bass_utils import run_bass_kernel_spmd

    if "nc" not in _CACHE:
        _CACHE["nc"] = _build_program()
    nc = _CACHE["nc"]

    in_maps = _make_in_maps(x, M1, M2)
    res = run_bass_kernel_spmd(nc, in_maps, core_ids=list(range(NCORES)))
    out = np.concatenate([res.results[r]["y"] for r in range(NCORES)], axis=0)
    return out.astype(np.float32)


# revision 12
# speedup vs baseline: 1.0126x; 1.0126x over previous
"""Trainium2 kernel for CompactBilinearLayer (count-sketch bilinear pooling).

Math: reference computes y = l2norm(signed_sqrt(sum_hw Re IFFT(FFT(x@M1)*FFT(x@M2)))).
Since M1/M2 are count-sketch matrices (one +-1 per row), FFT(x@M1) == x @ A1 with
A1[c,k] = s1[c] * exp(-2pi i h1[c] k / P) — a dense [512, K] matrix computable on the
host from M1 in O(C*K). The IFFT is linear, so the spatial sum moves before it.
Hermitian symmetry means only k = 0..4096 are needed.  Per core (4 batch elements,
784 spatial positions — fully batch-local, no collectives):
  A: P1/P2 projections = A^T @ x^T, single-pass bf16 matmuls; per-component
     [128,784] PSUM tiles with bank-aligned 512+272 splits; Act evacuates each
     component to SBUF so PSUM recycles fast and the PE stays fed
  B: S[k,b] = sum_t (P1*P2) per batch via fused DVE scalar_tensor_tensor
     (product+reduce in one op), operands all-SBUF
  C: IFFT via two-step factorization n=64q+s: GpSimd computes the twiddle
     products (f32r), PE accumulates 4 f32r matmuls over k%128 into psy.
     Stage-C matmuls are emitted 2 iterations late so the in-order PE queue
     never waits on the DVE->GpSimd chain.
  D: signed sqrt + per-batch L2 norm + store
"""
import numpy as np

P = 8192
C = 512
FT = 33            # frequency tiles of 128 -> 4224 slots >= 4097
NSLOT = FT * 128
NCORES = 8
BPC = 4            # batch elems per core
HW = 196           # spatial positions per batch elem
T = BPC * HW       # 784 positions per core
B = 32

_CACHE = {}


def _build_program():
    import concourse.bass as bass
    import concourse.tile as tile
    from concourse import bacc, mybir

    f32 = mybir.dt.float32
    f32r = mybir.dt.float32r
    bf16 = mybir.dt.bfloat16
    nc = bacc.Bacc("TRN2", target_bir_lowering=False, debug=False,
                   num_devices=NCORES)

    a_d = nc.dram_tensor("a", [FT, 128, 4, 512], bf16, kind="ExternalInput").ap()
    x_d = nc.dram_tensor("x", [128, 4, T], bf16, kind="ExternalInput").ap()
    cphi_d = nc.dram_tensor("cphi", [128, FT, 64], f32, kind="ExternalInput").ap()
    sphi_d = nc.dram_tensor("sphi", [128, FT, 64], f32, kind="ExternalInput").ap()
    cosa_d = nc.dram_tensor("cosa", [128, 128], f32r, kind="ExternalInput").ap()
    ncosa_d = nc.dram_tensor("ncosa", [128, 128], f32r, kind="ExternalInput").ap()
    nsina_d = nc.dram_tensor("nsina", [128, 128], f32r, kind="ExternalInput").ap()
    y_d = nc.dram_tensor("y", [BPC, P], f32, kind="ExternalOutput").ap()

    mult = mybir.AluOpType.mult
    bypass = mybir.AluOpType.bypass
    Act = mybir.ActivationFunctionType

    with tile.TileContext(nc) as tc:
        with (
            tc.tile_pool(name="const", bufs=1) as const,
            tc.tile_pool(name="apool", bufs=4) as apool,
            tc.tile_pool(name="pst", bufs=3, space="PSUM") as pstpool,
            tc.tile_pool(name="psyp", bufs=1, space="PSUM") as psypool,
            tc.tile_pool(name="scr", bufs=3) as scr,
            tc.tile_pool(name="uv", bufs=4) as uvpool,
        ):
            x_sb = const.tile([128, 4, T], bf16)
            nc.sync.dma_start(x_sb[:], x_d)
            cphi_sb = const.tile([128, FT, 64], f32)
            nc.sync.dma_start(cphi_sb[:], cphi_d)
            sphi_sb = const.tile([128, FT, 64], f32)
            nc.sync.dma_start(sphi_sb[:], sphi_d)
            cosa_sb = const.tile([128, 128], f32r)
            nc.sync.dma_start(cosa_sb[:], cosa_d)
            ncosa_sb = const.tile([128, 128], f32r)
            nc.sync.dma_start(ncosa_sb[:], ncosa_d)
            nsina_sb = const.tile([128, 128], f32r)
            nc.sync.dma_start(nsina_sb[:], nsina_d)
            ones_sb = const.tile([128, 1], f32)
            nc.vector.memset(ones_sb[:], 1.0)
            sre_sb = const.tile([128, FT * 4], f32)
            sim_sb = const.tile([128, FT * 4], f32)
            sA_sb = const.tile([128, FT * 4], f32)
            sB_sb = const.tile([128, FT * 4], f32)
            sC_sb = const.tile([128, FT * 4], f32)
            sD_sb = const.tile([128, FT * 4], f32)

            psy = psypool.tile([128, BPC * 64], f32, tag="psy")

            def emit_stage_c(ft, us):
                u1, u2, v1, v2 = us
                nc.tensor.matmul(psy[:], cosa_sb[:],
                                 u1[:].rearrange("p b s -> p (b s)"),
                                 start=(ft == 0), stop=False)
                nc.tensor.matmul(psy[:], ncosa_sb[:],
                                 u2[:].rearrange("p b s -> p (b s)"),
                                 start=False, stop=False)
                nc.tensor.matmul(psy[:], nsina_sb[:],
                                 v1[:].rearrange("p b s -> p (b s)"),
                                 start=False, stop=False)
                nc.tensor.matmul(psy[:], nsina_sb[:],
                                 v2[:].rearrange("p b s -> p (b s)"),
                                 start=False, stop=(ft == FT - 1))

            pend = {}
            for ft in range(FT):
                a_t = apool.tile([128, 4, 512], bf16, tag="a")
                nc.sync.dma_start(a_t[:], a_d[ft])
                psm = {}
                cpy = {}
                # components 2,3 first: Act evacuates them to SBUF (the stt
                # in1 operand); 0,1 stay in PSUM so each stt uses only one
                # SBUF read port and does not contend with GpSimd's port
                for m in (2, 3, 0, 1):
                    msl = slice(m * 128, (m + 1) * 128)
                    ps = pstpool.tile([128, T], f32, tag="pst",
                                      name=f"ps{m}_{ft}")
                    # bank-aligned output splits: 512 (bank 0), 272 (bank 1)
                    for c0, cn in ((0, 512), (512, T - 512)):
                        for ck in range(4):
                            nc.tensor.matmul(
                                ps[:, c0:c0 + cn],
                                a_t[:, ck, msl],
                                x_sb[:, ck, c0:c0 + cn],
                                start=(ck == 0),
                                stop=(ck == 3),
                            )
                    psm[m] = ps
                    if m in (2, 3):
                        c_m = scr.tile([128, T], f32, tag=f"c{m}",
                                       name=f"c{m}_{ft}")
                        nc.scalar.activation(c_m[:], ps[:], Act.Copy)
                        cpy[m] = c_m

                # A=sum p0*p2, B=sum p1*p3, C=sum p0*p3, D=sum p1*p2.
                # All ps0-reading ops first so ps0's PSUM ring slot frees
                # before the next tile's matmuls need it.
                for tg, (pa, cb, dst) in (
                    ("sc0", (psm[0], cpy[2], sA_sb)),
                    ("sc2", (psm[0], cpy[3], sC_sb)),
                    ("sc1", (psm[1], cpy[3], sB_sb)),
                    ("sc3", (psm[1], cpy[2], sD_sb)),
                ):
                    for bl in range(BPC):
                        idx = ft * 4 + bl
                        seg = slice(bl * HW, (bl + 1) * HW)
                        sc = scr.tile([128, HW], f32, tag=tg,
                                      name=f"{tg}_{ft}_{bl}")
                        nc.vector.scalar_tensor_tensor(
                            sc[:], pa[:, seg], 1.0, cb[:, seg],
                            bypass, mult,
                            accum_out=dst[:, idx:idx + 1])
                # ReS = A - B, ImS = C + D for this ft's 4 batch slots
                fsl = slice(ft * 4, (ft + 1) * 4)
                nc.vector.tensor_sub(sre_sb[:, fsl], sA_sb[:, fsl],
                                     sB_sb[:, fsl])
                nc.vector.tensor_add(sim_sb[:, fsl], sC_sb[:, fsl],
                                     sD_sb[:, fsl])

                # twiddle products on GpSimd (u = phi * S, broadcast both ways)
                u1 = uvpool.tile([128, BPC, 64], f32r, tag="u1", name=f"u1_{ft}")
                u2 = uvpool.tile([128, BPC, 64], f32r, tag="u2", name=f"u2_{ft}")
                v1 = uvpool.tile([128, BPC, 64], f32r, tag="v1", name=f"v1_{ft}")
                v2 = uvpool.tile([128, BPC, 64], f32r, tag="v2", name=f"v2_{ft}")
                cphb = cphi_sb[:, ft, :][:, None, :].broadcast_to([128, BPC, 64])
                sphb = sphi_sb[:, ft, :][:, None, :].broadcast_to([128, BPC, 64])
                sreb = sre_sb[:, fsl][:, :, None].broadcast_to([128, BPC, 64])
                simb = sim_sb[:, fsl][:, :, None].broadcast_to([128, BPC, 64])
                nc.gpsimd.tensor_tensor(u1[:], cphb, sreb, op=mult)
                nc.gpsimd.tensor_tensor(u2[:], sphb, simb, op=mult)
                nc.gpsimd.tensor_tensor(v1[:], sphb, sreb, op=mult)
                nc.gpsimd.tensor_tensor(v2[:], cphb, simb, op=mult)
                pend[ft] = (u1, u2, v1, v2)

                # emit IFFT matmuls 2 iterations late to keep the PE queue fed
                if ft >= 2:
                    emit_stage_c(ft - 2, pend.pop(ft - 2))
            emit_stage_c(FT - 2, pend.pop(FT - 2))
            emit_stage_c(FT - 1, pend.pop(FT - 1))

            # ---- stage D: signed sqrt, per-batch l2 norm, store ----
            absy = const.tile([128, BPC * 64], f32)
            nc.scalar.activation(absy[:], psy[:], Act.Abs)
            sqy = const.tile([128, BPC * 64], f32)
            nc.scalar.activation(sqy[:], absy[:], Act.Sqrt)
            sgn = const.tile([128, BPC * 64], f32)
            nc.scalar.activation(sgn[:], psy[:], Act.Sign)
            ys = const.tile([128, BPC * 64], f32)
            nc.vector.tensor_mul(ys[:], sqy[:], sgn[:])

            # norm^2 per batch = sum_p y^2 = sum_p |Y|  (Y = pre-sqrt value)
            psn = pstpool.tile([128, BPC * 64], f32, tag="pst", name="psn")
            nc.tensor.matmul(psn[0:1, :], ones_sb[:], absy[:],
                             start=True, stop=True)
            nsq = const.tile([1, BPC], f32)
            nc.vector.reduce_sum(
                out=nsq[:],
                in_=psn[0:1, :].rearrange("p (b s) -> p b s", b=BPC),
                axis=mybir.AxisListType.X,
            )
            nc.vector.tensor_scalar_max(nsq[:], nsq[:], 1e-10)
            sqn = const.tile([1, BPC], f32)
            nc.scalar.activation(sqn[:], nsq[:], Act.Sqrt)
            invn = const.tile([1, BPC], f32)
            nc.vector.reciprocal(invn[:], sqn[:])

            onesrow = const.tile([1, 128], f32)
            nc.vector.memset(onesrow[:], 1.0)
            psb = pstpool.tile([128, BPC], f32, tag="pst", name="psb")
            nc.tensor.matmul(psb[:, 0:BPC], onesrow[0:1, :], invn[0:1, :],
                             start=True, stop=True)
            inv_b = psb[:, 0:BPC][:, :, None].broadcast_to([128, BPC, 64])
            fin = const.tile([128, BPC * 64], f32)
            nc.vector.tensor_tensor(
                fin[:].rearrange("p (b s) -> p b s", b=BPC),
                ys[:].rearrange("p (b s) -> p b s", b=BPC),
                inv_b,
                op=mult,
            )
            for b in range(BPC):
                nc.sync.dma_start(
                    y_d[b].rearrange("(q s) -> q s", q=128),
                    fin[:, b * 64:(b + 1) * 64],
                )

    nc.compile()
    return nc


def _to_bf16(a):
    import ml_dtypes
    return np.asarray(a, np.float32).astype(ml_dtypes.bfloat16)


def _host_prep(x, M1, M2):
    x = np.ascontiguousarray(np.asarray(x, np.float32))
    M1 = np.asarray(M1, np.float32)
    M2 = np.asarray(M2, np.float32)

    h1 = np.argmax(np.abs(M1), axis=1)
    s1 = M1[np.arange(C), h1].astype(np.float64)
    h2 = np.argmax(np.abs(M2), axis=1)
    s2 = M2[np.arange(C), h2].astype(np.float64)

    k = np.arange(NSLOT, dtype=np.float64)
    valid = k <= P // 2
    ang1 = 2 * np.pi * np.outer(h1.astype(np.float64), k) / P
    ang2 = 2 * np.pi * np.outer(h2.astype(np.float64), k) / P
    # a[ft, c, m*128 + j]: m in (A1re, A1im, A2re, A2im), freq = ft*128 + j
    a = np.empty((FT, C, 512), np.float32)
    a1re = (s1[:, None] * np.cos(ang1) * valid).astype(np.float32)
    a1im = (-s1[:, None] * np.sin(ang1) * valid).astype(np.float32)
    a2re = (s2[:, None] * np.cos(ang2) * valid).astype(np.float32)
    a2im = (-s2[:, None] * np.sin(ang2) * valid).astype(np.float32)
    for ft in range(FT):
        ksl = slice(ft * 128, (ft + 1) * 128)
        a[ft, :, 0:128] = a1re[:, ksl]
        a[ft, :, 128:256] = a1im[:, ksl]
        a[ft, :, 256:384] = a2re[:, ksl]
        a[ft, :, 384:512] = a2im[:, ksl]

    w = np.where(valid, 2.0 / P, 0.0)
    w[0] = 1.0 / P
    w[P // 2] = 1.0 / P
    s_idx = np.arange(64, dtype=np.float64)
    phi = 2 * np.pi * np.outer(k, s_idx) / P
    cphi = (w[:, None] * np.cos(phi)).astype(np.float32).reshape(FT, 128, 64)
    sphi = (w[:, None] * np.sin(phi)).astype(np.float32).reshape(FT, 128, 64)

    km = np.arange(128, dtype=np.float64)
    alpha = 2 * np.pi * np.outer(km, km) / 128
    cosa = np.cos(alpha).astype(np.float32)
    nsina = (-np.sin(alpha)).astype(np.float32)

    xt = np.ascontiguousarray(x.reshape(B * HW, C).T)  # [C, 6272]

    # pre-transposed, DMA-contiguous layouts
    a_t = np.ascontiguousarray(
        a.reshape(FT, 4, 128, 512).transpose(0, 2, 1, 3))   # [FT,128p,4ck,512m]
    cphi_t = np.ascontiguousarray(cphi.transpose(1, 0, 2))  # [128p,FT,64]
    sphi_t = np.ascontiguousarray(sphi.transpose(1, 0, 2))
    xt_t = xt.reshape(4, 128, B * HW).transpose(1, 0, 2)    # [128p,4ck,T]
    return _to_bf16(a_t), cphi_t, sphi_t, cosa, -cosa, nsina, _to_bf16(xt_t)


def _make_in_maps(x, M1, M2):
    a, cphi, sphi, cosa, ncosa, nsina, xt = _host_prep(x, M1, M2)
    in_maps = []
    for r in range(NCORES):
        in_maps.append({
            "a": a,
            "x": np.ascontiguousarray(xt[:, :, r * T:(r + 1) * T]),
            "cphi": cphi,
            "sphi": sphi,
            "cosa": cosa,
            "ncosa": ncosa,
            "nsina": nsina,
        })
    return in_maps


def kernel(x, M1, M2):
    from concourse.bass_utils import run_bass_kernel_spmd

    if "nc" not in _CACHE:
        _CACHE["nc"] = _build_program()
    nc = _CACHE["nc"]

    in_maps = _make_in_maps(x, M1, M2)
    res = run_bass_kernel_spmd(nc, in_maps, core_ids=list(range(NCORES)))
    out = np.concatenate([res.results[r]["y"] for r in range(NCORES)], axis=0)
    return out.astype(np.float32)


# revision 13
# speedup vs baseline: 1.0433x; 1.0303x over previous
"""Trainium2 kernel for CompactBilinearLayer (count-sketch bilinear pooling).

Math: reference computes y = l2norm(signed_sqrt(sum_hw Re IFFT(FFT(x@M1)*FFT(x@M2)))).
Since M1/M2 are count-sketch matrices (one +-1 per row), FFT(x@M1) == x @ A1 with
A1[c,k] = s1[c] * exp(-2pi i h1[c] k / P) — a dense [512, K] matrix computable on the
host from M1 in O(C*K). The IFFT is linear, so the spatial sum moves before it.
Hermitian symmetry means only k = 0..4096 are needed.  Per core (4 batch elements,
784 spatial positions — fully batch-local, no collectives):
  A: P1/P2 projections = A^T @ x^T, single-pass bf16 matmuls; per-component
     [128,784] PSUM tiles with bank-aligned 512+272 splits; Act evacuates each
     component to SBUF so PSUM recycles fast and the PE stays fed
  B: S[k,b] = sum_t (P1*P2) per batch via fused DVE scalar_tensor_tensor
     (product+reduce in one op), operands all-SBUF
  C: IFFT via two-step factorization n=64q+s: GpSimd computes the twiddle
     products (f32r), PE accumulates 4 f32r matmuls over k%128 into psy.
     Stage-C matmuls are emitted 2 iterations late so the in-order PE queue
     never waits on the DVE->GpSimd chain.
  D: signed sqrt + per-batch L2 norm + store
"""
import numpy as np

P = 8192
C = 512
FT = 33            # frequency tiles of 128 -> 4224 slots >= 4097
NSLOT = FT * 128
NCORES = 8
BPC = 4            # batch elems per core
HW = 196           # spatial positions per batch elem
T = BPC * HW       # 784 positions per core
B = 32

_CACHE = {}


def _build_program():
    import concourse.bass as bass
    import concourse.tile as tile
    from concourse import bacc, mybir

    f32 = mybir.dt.float32
    f32r = mybir.dt.float32r
    bf16 = mybir.dt.bfloat16
    nc = bacc.Bacc("TRN2", target_bir_lowering=False, debug=False,
                   num_devices=NCORES)

    a_d = nc.dram_tensor("a", [FT, 128, 4, 512], bf16, kind="ExternalInput").ap()
    x_d = nc.dram_tensor("x", [128, 4, T], bf16, kind="ExternalInput").ap()
    cphi_d = nc.dram_tensor("cphi", [128, FT, 64], f32, kind="ExternalInput").ap()
    sphi_d = nc.dram_tensor("sphi", [128, FT, 64], f32, kind="ExternalInput").ap()
    cosa_d = nc.dram_tensor("cosa", [128, 128], f32r, kind="ExternalInput").ap()
    ncosa_d = nc.dram_tensor("ncosa", [128, 128], f32r, kind="ExternalInput").ap()
    nsina_d = nc.dram_tensor("nsina", [128, 128], f32r, kind="ExternalInput").ap()
    y_d = nc.dram_tensor("y", [BPC, P], f32, kind="ExternalOutput").ap()

    mult = mybir.AluOpType.mult
    bypass = mybir.AluOpType.bypass
    Act = mybir.ActivationFunctionType

    with tile.TileContext(nc) as tc:
        with (
            tc.tile_pool(name="const", bufs=1) as const,
            tc.tile_pool(name="apool", bufs=6) as apool,
            tc.tile_pool(name="pst", bufs=3, space="PSUM") as pstpool,
            tc.tile_pool(name="psyp", bufs=1, space="PSUM") as psypool,
            tc.tile_pool(name="scr", bufs=3) as scr,
            tc.tile_pool(name="uv", bufs=4) as uvpool,
        ):
            x_sb = const.tile([128, 4, T], bf16)
            nc.sync.dma_start(x_sb[:], x_d)
            cphi_sb = const.tile([128, FT, 64], f32)
            sphi_sb = const.tile([128, FT, 64], f32)
            cosa_sb = const.tile([128, 128], f32r)
            ncosa_sb = const.tile([128, 128], f32r)
            nsina_sb = const.tile([128, 128], f32r)
            ones_sb = const.tile([128, 1], f32)
            nc.vector.memset(ones_sb[:], 1.0)
            # preload the Abs/Sqrt/Sign activation tables during the initial
            # DMA window so stage D doesn't pay the ~2.6us table switch
            warm = const.tile([1, 1], f32)
            nc.vector.memset(warm[:], 1.0)
            wo = const.tile([1, 1], f32)
            nc.scalar.activation(wo[:], warm[:], Act.Abs)
            nc.scalar.activation(wo[:], wo[:], Act.Sqrt)
            nc.scalar.activation(wo[:], wo[:], Act.Sign)
            sre_sb = const.tile([128, FT * 4], f32)
            sim_sb = const.tile([128, FT * 4], f32)
            sA_sb = const.tile([128, FT * 4], f32)
            sB_sb = const.tile([128, FT * 4], f32)
            sC_sb = const.tile([128, FT * 4], f32)
            sD_sb = const.tile([128, FT * 4], f32)

            psy = psypool.tile([128, BPC * 64], f32, tag="psy")

            def emit_stage_c(ft, us):
                u1, u2, v1, v2 = us
                nc.tensor.matmul(psy[:], cosa_sb[:],
                                 u1[:].rearrange("p b s -> p (b s)"),
                                 start=(ft == 0), stop=False)
                nc.tensor.matmul(psy[:], ncosa_sb[:],
                                 u2[:].rearrange("p b s -> p (b s)"),
                                 start=False, stop=False)
                nc.tensor.matmul(psy[:], nsina_sb[:],
                                 v1[:].rearrange("p b s -> p (b s)"),
                                 start=False, stop=False)
                nc.tensor.matmul(psy[:], nsina_sb[:],
                                 v2[:].rearrange("p b s -> p (b s)"),
                                 start=False, stop=(ft == FT - 1))

            pend = {}
            for ft in range(FT):
                a_t = apool.tile([128, 4, 512], bf16, tag="a")
                nc.sync.dma_start(a_t[:], a_d[ft])
                if ft == 0:
                    nc.sync.dma_start(cphi_sb[:], cphi_d)
                    nc.sync.dma_start(sphi_sb[:], sphi_d)
                    nc.sync.dma_start(cosa_sb[:], cosa_d)
                    nc.sync.dma_start(ncosa_sb[:], ncosa_d)
                    nc.sync.dma_start(nsina_sb[:], nsina_d)
                psm = {}
                cpy = {}
                # components 2,3 first: Act evacuates them to SBUF (the stt
                # in1 operand); 0,1 stay in PSUM so each stt uses only one
                # SBUF read port and does not contend with GpSimd's port
                for m in (2, 3, 0, 1):
                    msl = slice(m * 128, (m + 1) * 128)
                    ps = pstpool.tile([128, T], f32, tag="pst",
                                      name=f"ps{m}_{ft}")
                    # bank-aligned output splits: 512 (bank 0), 272 (bank 1)
                    for c0, cn in ((0, 512), (512, T - 512)):
                        for ck in range(4):
                            nc.tensor.matmul(
                                ps[:, c0:c0 + cn],
                                a_t[:, ck, msl],
                                x_sb[:, ck, c0:c0 + cn],
                                start=(ck == 0),
                                stop=(ck == 3),
                            )
                    psm[m] = ps
                    if m in (2, 3):
                        c_m = scr.tile([128, T], f32, tag=f"c{m}",
                                       name=f"c{m}_{ft}")
                        nc.scalar.activation(c_m[:], ps[:], Act.Copy)
                        cpy[m] = c_m

                # A=sum p0*p2, B=sum p1*p3, C=sum p0*p3, D=sum p1*p2.
                # All ps0-reading ops first so ps0's PSUM ring slot frees
                # before the next tile's matmuls need it.
                for tg, (pa, cb, dst) in (
                    ("sc0", (psm[0], cpy[2], sA_sb)),
                    ("sc2", (psm[0], cpy[3], sC_sb)),
                    ("sc1", (psm[1], cpy[3], sB_sb)),
                    ("sc3", (psm[1], cpy[2], sD_sb)),
                ):
                    for bl in range(BPC):
                        idx = ft * 4 + bl
                        seg = slice(bl * HW, (bl + 1) * HW)
                        sc = scr.tile([128, HW], f32, tag=tg,
                                      name=f"{tg}_{ft}_{bl}")
                        nc.vector.scalar_tensor_tensor(
                            sc[:], pa[:, seg], 1.0, cb[:, seg],
                            bypass, mult,
                            accum_out=dst[:, idx:idx + 1])
                # ReS = A - B, ImS = C + D for this ft's 4 batch slots
                fsl = slice(ft * 4, (ft + 1) * 4)
                nc.vector.tensor_sub(sre_sb[:, fsl], sA_sb[:, fsl],
                                     sB_sb[:, fsl])
                nc.vector.tensor_add(sim_sb[:, fsl], sC_sb[:, fsl],
                                     sD_sb[:, fsl])

                # twiddle products on GpSimd (u = phi * S, broadcast both ways)
                u1 = uvpool.tile([128, BPC, 64], f32r, tag="u1", name=f"u1_{ft}")
                u2 = uvpool.tile([128, BPC, 64], f32r, tag="u2", name=f"u2_{ft}")
                v1 = uvpool.tile([128, BPC, 64], f32r, tag="v1", name=f"v1_{ft}")
                v2 = uvpool.tile([128, BPC, 64], f32r, tag="v2", name=f"v2_{ft}")
                cphb = cphi_sb[:, ft, :][:, None, :].broadcast_to([128, BPC, 64])
                sphb = sphi_sb[:, ft, :][:, None, :].broadcast_to([128, BPC, 64])
                sreb = sre_sb[:, fsl][:, :, None].broadcast_to([128, BPC, 64])
                simb = sim_sb[:, fsl][:, :, None].broadcast_to([128, BPC, 64])
                nc.gpsimd.tensor_tensor(u1[:], cphb, sreb, op=mult)
                nc.gpsimd.tensor_tensor(u2[:], sphb, simb, op=mult)
                nc.gpsimd.tensor_tensor(v1[:], sphb, sreb, op=mult)
                nc.gpsimd.tensor_tensor(v2[:], cphb, simb, op=mult)
                pend[ft] = (u1, u2, v1, v2)

                # emit IFFT matmuls 2 iterations late to keep the PE queue fed
                if ft >= 2:
                    emit_stage_c(ft - 2, pend.pop(ft - 2))
            emit_stage_c(FT - 2, pend.pop(FT - 2))
            emit_stage_c(FT - 1, pend.pop(FT - 1))

            # ---- stage D: signed sqrt, per-batch l2 norm, store ----
            absy = const.tile([128, BPC * 64], f32)
            nc.scalar.activation(absy[:], psy[:], Act.Abs)
            sqy = const.tile([128, BPC * 64], f32)
            nc.scalar.activation(sqy[:], absy[:], Act.Sqrt)
            sgn = const.tile([128, BPC * 64], f32)
            nc.scalar.activation(sgn[:], psy[:], Act.Sign)
            ys = const.tile([128, BPC * 64], f32)
            nc.vector.tensor_mul(ys[:], sqy[:], sgn[:])

            # norm^2 per batch = sum_p y^2 = sum_p |Y|  (Y = pre-sqrt value)
            psn = pstpool.tile([128, BPC * 64], f32, tag="pst", name="psn")
            nc.tensor.matmul(psn[0:1, :], ones_sb[:], absy[:],
                             start=True, stop=True)
            nsq = const.tile([1, BPC], f32)
            nc.vector.reduce_sum(
                out=nsq[:],
                in_=psn[0:1, :].rearrange("p (b s) -> p b s", b=BPC),
                axis=mybir.AxisListType.X,
            )
            nc.vector.tensor_scalar_max(nsq[:], nsq[:], 1e-10)
            sqn = const.tile([1, BPC], f32)
            nc.scalar.activation(sqn[:], nsq[:], Act.Sqrt)
            invn = const.tile([1, BPC], f32)
            nc.vector.reciprocal(invn[:], sqn[:])

            onesrow = const.tile([1, 128], f32)
            nc.vector.memset(onesrow[:], 1.0)
            psb = pstpool.tile([128, BPC], f32, tag="pst", name="psb")
            nc.tensor.matmul(psb[:, 0:BPC], onesrow[0:1, :], invn[0:1, :],
                             start=True, stop=True)
            inv_b = psb[:, 0:BPC][:, :, None].broadcast_to([128, BPC, 64])
            fin = const.tile([128, BPC * 64], f32)
            nc.vector.tensor_tensor(
                fin[:].rearrange("p (b s) -> p b s", b=BPC),
                ys[:].rearrange("p (b s) -> p b s", b=BPC),
                inv_b,
                op=mult,
            )
            for b in range(BPC):
                nc.sync.dma_start(
                    y_d[b].rearrange("(q s) -> q s", q=128),
                    fin[:, b * 64:(b + 1) * 64],
                )

    nc.compile()
    return nc


def _to_bf16(a):
    import ml_dtypes
    return np.asarray(a, np.float32).astype(ml_dtypes.bfloat16)


def _host_prep(x, M1, M2):
    x = np.ascontiguousarray(np.asarray(x, np.float32))
    M1 = np.asarray(M1, np.float32)
    M2 = np.asarray(M2, np.float32)

    h1 = np.argmax(np.abs(M1), axis=1)
    s1 = M1[np.arange(C), h1].astype(np.float64)
    h2 = np.argmax(np.abs(M2), axis=1)
    s2 = M2[np.arange(C), h2].astype(np.float64)

    k = np.arange(NSLOT, dtype=np.float64)
    valid = k <= P // 2
    ang1 = 2 * np.pi * np.outer(h1.astype(np.float64), k) / P
    ang2 = 2 * np.pi * np.outer(h2.astype(np.float64), k) / P
    # a[ft, c, m*128 + j]: m in (A1re, A1im, A2re, A2im), freq = ft*128 + j
    a = np.empty((FT, C, 512), np.float32)
    a1re = (s1[:, None] * np.cos(ang1) * valid).astype(np.float32)
    a1im = (-s1[:, None] * np.sin(ang1) * valid).astype(np.float32)
    a2re = (s2[:, None] * np.cos(ang2) * valid).astype(np.float32)
    a2im = (-s2[:, None] * np.sin(ang2) * valid).astype(np.float32)
    for ft in range(FT):
        ksl = slice(ft * 128, (ft + 1) * 128)
        a[ft, :, 0:128] = a1re[:, ksl]
        a[ft, :, 128:256] = a1im[:, ksl]
        a[ft, :, 256:384] = a2re[:, ksl]
        a[ft, :, 384:512] = a2im[:, ksl]

    w = np.where(valid, 2.0 / P, 0.0)
    w[0] = 1.0 / P
    w[P // 2] = 1.0 / P
    s_idx = np.arange(64, dtype=np.float64)
    phi = 2 * np.pi * np.outer(k, s_idx) / P
    cphi = (w[:, None] * np.cos(phi)).astype(np.float32).reshape(FT, 128, 64)
    sphi = (w[:, None] * np.sin(phi)).astype(np.float32).reshape(FT, 128, 64)

    km = np.arange(128, dtype=np.float64)
    alpha = 2 * np.pi * np.outer(km, km) / 128
    cosa = np.cos(alpha).astype(np.float32)
    nsina = (-np.sin(alpha)).astype(np.float32)

    xt = np.ascontiguousarray(x.reshape(B * HW, C).T)  # [C, 6272]

    # pre-transposed, DMA-contiguous layouts
    a_t = np.ascontiguousarray(
        a.reshape(FT, 4, 128, 512).transpose(0, 2, 1, 3))   # [FT,128p,4ck,512m]
    cphi_t = np.ascontiguousarray(cphi.transpose(1, 0, 2))  # [128p,FT,64]
    sphi_t = np.ascontiguousarray(sphi.transpose(1, 0, 2))
    xt_t = xt.reshape(4, 128, B * HW).transpose(1, 0, 2)    # [128p,4ck,T]
    return _to_bf16(a_t), cphi_t, sphi_t, cosa, -cosa, nsina, _to_bf16(xt_t)


def _make_in_maps(x, M1, M2):
    a, cphi, sphi, cosa, ncosa, nsina, xt = _host_prep(x, M1, M2)
    in_maps = []
    for r in range(NCORES):
        in_maps.append({
            "a": a,
            "x": np.ascontiguousarray(xt[:, :, r * T:(r + 1) * T]),
            "cphi": cphi,
            "sphi": sphi,
            "cosa": cosa,
            "ncosa": ncosa,
            "nsina": nsina,
        })
    return in_maps


def kernel(x, M1, M2):
    from concourse.bass_utils import run_bass_kernel_spmd

    if "nc" not in _CACHE:
        _CACHE["nc"] = _build_program()
    nc = _CACHE["nc"]

    in_maps = _make_in_maps(x, M1, M2)
    res = run_bass_kernel_spmd(nc, in_maps, core_ids=list(range(NCORES)))
    out = np.concatenate([res.results[r]["y"] for r in range(NCORES)], axis=0)
    return out.astype(np.float32)
